# revision 26
# baseline (speedup 1.0000x reference)
"""AnyPrecisionLinear (4-bit LUT-quantized linear) on 8 TRN2 NeuronCores.

Reference computes:  out = x @ W.T,  W[o,i] = lut[o, qweight[o,i]]
  x: [64, 8192] fp16, qweight: [8192, 8192] int32 (values 0..15),
  lut: [8192, 16] fp16  ->  out: [64, 8192] fp16

Strategy (tensor-parallel along out_features, per the sharding hint):
  * Host re-encodes each row's 16-entry LUT affine into uint8 codes
    (scale s[o], offset mn[o]); ships a [8192, 1024] uint8 code shard
    per core (1 B/weight).
  * Device: streams code groups, casts uint8->fp16 on DVE+ACT in
    parallel, accumulates x @ codes.T on PE as two concurrent
    column-tiled chains (PSUM partitions 0-63 / 64-127).
  * mn[o]*xsum fold via rank-1 matmuls closing each chain; chain A
    closes first so its scale-epilogue + out-DMA overlap chain B's
    final matmuls. Host concatenates/reshapes.

Schedule (from NTFF trace analysis; best-measured variant):
  * Fine-grained single-queue DMA: 256KB first pieces so casting
    starts ~10.5us, x shipped in three k-range pieces (xsb_a covers
    groups 0-3, xsb_m 4-9, xsb_c 10-15 + xsum row + mnr + scales as
    fp16), weight tail split 2x256KB across both engines.
  * Static DVE/ACT cast assignment derived from an embedded
    event-driven list-scheduling model of arrivals.
  * Long PE warmup (~150 matmuls) bridges body-start through the
    recurring early-stream stall into the first real matmul, so the
    PE stays backlogged behind the casts for the whole kernel - it
    never idles >3us, which would trip the HAM duty-cycle throttle
    (half PE clock for a 7-10us window).
"""

import numpy as np

import concourse.bass as bass
from concourse import bacc, mybir
from concourse.bass_utils import run_bass_kernel_spmd

B, IN, OUT, NCORES = 64, 8192, 8192, 8
OSH = OUT // NCORES          # 1024 output columns per core
KT = IN // 128               # 64 contraction k-tiles of 128
NG = 16                      # 512KB cast groups (4 k-tiles each)
KPG = KT // NG               # k-tiles per group (4)
GSZ = KPG * OSH              # free elems per group (4096)

# x SBUF image pieces (free-elem offsets into the xtm tensor)
XA_KT, XM_KT = 16, 40        # xsb_a covers kt 0-15, xsb_m 16-39, xsb_c rest
XA_E = XA_KT * B                       # 1024
XM_E = (XM_KT - XA_KT) * B             # 1536
XSUM_OFF = KT * B                      # 4096 (xsum/16 row in tile KT)
MNR_OFF = (KT + 1) * B                 # 4160
SB_OFF = MNR_OFF + OSH                 # 5184
XTM_FREE = SB_OFF + 512                # 5696 fp16 elems per partition

WARMUP = 150                 # PE warmup matmuls ([128,32]x[128,128])
N_SLOTS = 8                  # rotating fp16 cast-group buffers
SPLIT_GROUPS = (0, 15)       # groups DMAd/cast as 2x256KB halves

_cached_nc = None
_last_in_maps = None


def _schedule():
    """Derive the DMA order and static DVE/ACT cast assignment from an
    event-driven model of the single-queue stream (~425 GB/s measured).
    Returns (dma_order, piece_assign, piece_order, piece_group).
    """
    XFER0, BW, GAP, SEM = 9.15, 0.43, 0.05, 0.93
    DVE_U, ACT_U, DISP = 4.60, 7.40, 0.08   # us per MB of uint8

    pieces = []          # (name, group, MB)
    for g in range(NG):
        if g in SPLIT_GROUPS:
            pieces += [(f'g{g}a', g, 0.25), (f'g{g}b', g, 0.25)]
        else:
            pieces.append((f'g{g}', g, 0.5))
    sizes = {n: mb for n, _, mb in pieces}
    sizes.update({'xsb_a': 0.26, 'xsb_m': 0.40, 'xsb_c': 0.79})

    dma_order = []
    for g in range(NG):
        dma_order += [n for n, gg, _ in pieces if gg == g]
        if g == 0:
            dma_order.append('xsb_a')
        if g == 3:
            dma_order.append('xsb_m')
        if g == 9:
            dma_order.append('xsb_c')

    t = XFER0
    arr = {}
    for n in dma_order:
        t += sizes[n] / BW + GAP
        arr[n] = t + SEM

    free = {'D': 9.3, 'A': 8.6}
    rate = {'D': DVE_U, 'A': ACT_U}
    assign, done = {}, {}
    left = [n for n, _, _ in pieces]
    while left:
        e = min(free, key=lambda k: free[k])
        cand = min(left, key=lambda n: arr[n])
        o = 'A' if e == 'D' else 'D'
        fin_e = max(free[e], arr[cand]) + DISP + sizes[cand] * rate[e]
        fin_o = max(free[o], arr[cand]) + DISP + sizes[cand] * rate[o]
        if len(left) <= 3 and fin_o < fin_e:
            e, fin_e = o, fin_o
        free[e] = fin_e
        done[cand] = fin_e
        assign[cand] = e
        left.remove(cand)
    order = sorted(done, key=lambda n: done[n])
    return dma_order, assign, order, {n: g for n, g, _ in pieces}


def _build():
    global _cached_nc
    if _cached_nc is not None:
        return _cached_nc
    from contextlib import ExitStack

    dma_order, assign, piece_order, piece_group = _schedule()

    nc = bacc.Bacc(
        "TRN2",
        target_bir_lowering=False,
        debug=False,
        enable_asserts=False,
        num_devices=NCORES,
    )
    xsb = nc.dram_tensor("xsb", [128, XTM_FREE], mybir.dt.float16, kind="ExternalInput")
    w8 = nc.dram_tensor("w8", [128, KT * OSH], mybir.dt.uint8, kind="ExternalInput")
    out = nc.dram_tensor("out", [128, 512], mybir.dt.float16, kind="ExternalOutput")

    # free-elem ranges for each DMA piece
    def piece_rng(name):
        if name == 'xsb_a':
            return 'x', 0, XA_E
        if name == 'xsb_m':
            return 'x', XA_E, XA_E + XM_E
        if name == 'xsb_c':
            return 'x', XA_E + XM_E, XTM_FREE
        g = piece_group[name]
        lo = g * GSZ
        hi = lo + GSZ
        if name.endswith('a'):
            hi = lo + GSZ // 2
        elif name.endswith('b'):
            lo = lo + GSZ // 2
        return 'w', lo, hi

    # cast ordinal per engine (sem thresholds)
    dord, aord = {}, {}
    for n in piece_order:
        if assign[n] == 'D':
            dord[n] = len(dord) + 1
        else:
            aord[n] = len(aord) + 1

    with ExitStack() as ctx:
        ec = ctx.enter_context
        dsems = {n: ec(nc.semaphore(f"d_{n}")) for n in dma_order}
        dum = ec(nc.semaphore("dum"))
        wzs = ec(nc.semaphore("wzs"))
        dcast = ec(nc.semaphore("dcast"))
        acast = ec(nc.semaphore("acast"))
        mmp = ec(nc.semaphore("mmp"))      # groups consumed; +NG: A closed, +NG+1: B
        epiA = ec(nc.semaphore("epiA"))
        epiB = ec(nc.semaphore("epiB"))
        doutA = ec(nc.semaphore("doutA"))
        doutB = ec(nc.semaphore("doutB"))
        xtm = ec(nc.sbuf_tensor("xtm", [128, XTM_FREE], mybir.dt.float16))
        w8t = ec(nc.sbuf_tensor("w8t", [128, KT * OSH], mybir.dt.uint8))
        wf = ec(nc.sbuf_tensor("wf", [128, N_SLOTS * GSZ], mybir.dt.float16))
        o16 = ec(nc.sbuf_tensor("o16", [128, 512], mybir.dt.float16))
        wz = ec(nc.sbuf_tensor("wz", [128, 128], mybir.dt.float16))
        ps1 = ec(nc.psum_tensor("ps1", [128, 512], mybir.dt.float32))
        ps2 = ec(nc.psum_tensor("ps2", [128, 512], mybir.dt.float32))
        wps = ec(nc.psum_tensor("wps", [32, 128], mybir.dt.float32))
        block = ec(nc.Block())

        # Exact ring-slot recycling thresholds: PE consumes pieces in the
        # static piece_order, so the group-completion sequence is known at
        # build time. Cast of group g (slot g % N_SLOTS) must wait until
        # group g - N_SLOTS has been fully consumed; mmp counts completed
        # groups in completion order.
        grp_completion = []
        seen = set()
        for n in piece_order:
            seen.add(n)
            g = piece_group[n]
            if g not in grp_completion and all(
                pn in seen for pn in piece_group if piece_group[pn] == g
            ):
                grp_completion.append(g)

        def slot_wait(g):
            prev = g - N_SLOTS
            if prev < 0:
                return 0
            return grp_completion.index(prev) + 1

        # groups fully consumed before the last piece (for epilogue gating)
        last_piece = piece_order[-1]
        n_grps_before_last = len(grp_completion) - (
            1 if piece_group[last_piece] == grp_completion[-1] else 0
        )

        @block.sync
        def _(sync):
            for n in dma_order:
                kind, lo, hi = piece_rng(n)
                if kind == 'x':
                    sync.dma_start(xtm[:, lo:hi], xsb[:, lo:hi]).then_inc(dsems[n], 16)
                else:
                    sync.dma_start(w8t[:, lo:hi], w8[:, lo:hi]).then_inc(dsems[n], 16)
            # Trailing dummy transfers (into the long-consumed group-0 uint8
            # region): keep the queue non-empty so the REAL tail pieces'
            # final descriptor groups don't straggle (+2.3us observed when
            # the queue drains right at the critical last arrivals).
            sync.dma_start(w8t[:, 0:4096], w8[:, 0:4096]).then_inc(dum, 16)
            sync.dma_start(w8t[:, 0:4096], w8[:, 0:4096]).then_inc(dum, 16)
            sync.wait_ge(epiA, 1)
            sync.dma_start(out[0:64, :], o16[0:64, :]).then_inc(doutA, 16)
            sync.wait_ge(epiB, 1)
            sync.dma_start(out[64:128, :], o16[64:128, :]).then_inc(doutB, 16)
            sync.wait_ge(doutA, 16)
            sync.wait_ge(doutB, 16)
            sync.wait_ge(dum, 32)

        @block.vector
        def _(vector):
            vector.memset(wz[:, :], 0).then_inc(wzs, 1)
            for n in piece_order:
                if assign[n] != 'D':
                    continue
                g = piece_group[n]
                _, lo, hi = piece_rng(n)
                sl = (g % N_SLOTS) * GSZ + (lo - g * GSZ)
                vector.wait_ge(dsems[n], 16)
                w = slot_wait(g)
                if w:
                    vector.wait_ge(mmp, w)
                vector.tensor_copy(
                    wf[:, sl : sl + (hi - lo)], w8t[:, lo:hi]
                ).then_inc(dcast, 1)
            # epilogue: half A as soon as chain A closes (its out-DMA then
            # overlaps the half-B mul which waits for chain B)
            vector.wait_ge(mmp, n_grps_before_last + 1)   # rank1-A fired
            vector.wait_ge(dsems['xsb_c'], 16)
            vector.tensor_mul(
                o16[0:64, :], ps1[0:64, :], xtm[0:64, SB_OFF : SB_OFF + 512]
            ).then_inc(epiA, 1)
            vector.wait_ge(mmp, n_grps_before_last + 2)   # rank1-B fired
            vector.tensor_mul(
                o16[64:128, :], ps2[64:128, :], xtm[64:128, SB_OFF : SB_OFF + 512]
            ).then_inc(epiB, 1)

        @block.scalar
        def _(scalar):
            for n in piece_order:
                if assign[n] != 'A':
                    continue
                g = piece_group[n]
                _, lo, hi = piece_rng(n)
                sl = (g % N_SLOTS) * GSZ + (lo - g * GSZ)
                scalar.wait_ge(dsems[n], 16)
                w = slot_wait(g)
                if w:
                    scalar.wait_ge(mmp, w)
                scalar.copy(
                    wf[:, sl : sl + (hi - lo)], w8t[:, lo:hi]
                ).then_inc(acast, 1)

        @block.tensor
        def _(tensor):
            tensor.wait_ge(wzs, 1)
            for _ in range(WARMUP):
                tensor.matmul(wps.ap(), wz[:, 0:32], wz[:, :], start=True, stop=True)
            psA = ps1[0:64, :]
            psB = ps2[64:128, :]
            xs_lhs = xtm[:, XSUM_OFF : XSUM_OFF + B]
            # pieces in expected completion order; accumulation order is free
            consumed = set()
            waited_gates = set()
            first = True
            for n in piece_order:
                g = piece_group[n]
                _, lo, hi = piece_rng(n)
                sl = (g % N_SLOTS) * GSZ + (lo - g * GSZ)
                if assign[n] == 'D':
                    tensor.wait_ge(dcast, dord[n])
                else:
                    tensor.wait_ge(acast, aord[n])
                xgate = 'xsb_a' if g <= 3 else ('xsb_m' if g <= 9 else 'xsb_c')
                if xgate not in waited_gates:
                    tensor.wait_ge(dsems[xgate], 16)
                    waited_gates.add(xgate)
                nkt = (hi - lo) // OSH
                k0 = lo // OSH
                if n == piece_order[-1]:
                    if 'xsb_c' not in waited_gates:
                        tensor.wait_ge(dsems['xsb_c'], 16)
                    # de-interleaved tail: close chain A first (rank1-A), so
                    # the A epilogue + out-DMA overlap chain B's final mms
                    for j in range(nkt):
                        k = k0 + j
                        lhsT = xtm[:, k * B : (k + 1) * B]
                        rhs = wf[:, sl + j * OSH : sl + (j + 1) * OSH]
                        tensor.matmul(psA, lhsT, rhs[:, 0:512], start=False, stop=False)
                    tensor.matmul(
                        psA, xs_lhs, xtm[:, MNR_OFF : MNR_OFF + 512],
                        start=False, stop=True,
                    ).then_inc(mmp, 1)
                    for j in range(nkt):
                        k = k0 + j
                        lhsT = xtm[:, k * B : (k + 1) * B]
                        rhs = wf[:, sl + j * OSH : sl + (j + 1) * OSH]
                        tensor.matmul(psB, lhsT, rhs[:, 512:1024], start=False, stop=False)
                    tensor.matmul(
                        psB, xs_lhs, xtm[:, MNR_OFF + 512 : MNR_OFF + 1024],
                        start=False, stop=True,
                    ).then_inc(mmp, 1)
                    continue
                last_mm = None
                for j in range(nkt):
                    k = k0 + j
                    lhsT = xtm[:, k * B : (k + 1) * B]
                    rhs = wf[:, sl + j * OSH : sl + (j + 1) * OSH]
                    tensor.matmul(psA, lhsT, rhs[:, 0:512], start=first, stop=False)
                    last_mm = tensor.matmul(
                        psB, lhsT, rhs[:, 512:1024], start=first, stop=False
                    )
                    first = False
                consumed.add(n)
                grp_done = all(
                    (pn in consumed) for pn in piece_group if piece_group[pn] == g
                )
                if grp_done:
                    last_mm.then_inc(mmp, 1)

    nc.compile()
    _cached_nc = nc
    return nc


def kernel(x, qweight, lut):
    x = np.asarray(x, dtype=np.float16)
    qweight = np.asarray(qweight, dtype=np.int32)
    lut = np.asarray(lut, dtype=np.float16)

    # Per-row affine re-encode of the LUT into uint8 codes.
    lut32 = lut.astype(np.float32)
    mn = lut32.min(axis=1)
    mx_ = lut32.max(axis=1)
    rng = mx_ - mn
    rng[rng == 0] = 1.0
    s = (rng / 255.0).astype(np.float32)               # [OUT]
    lutcodes = np.rint((lut32 - mn[:, None]) * (255.0 / rng)[:, None]).astype(np.uint8)
    codes = np.take_along_axis(lutcodes, qweight, axis=1)  # [OUT, IN] uint8

    # x SBUF image: [128, XTM_FREE] fp16
    #   cols [0, KT*B): x tiles (partition k%128, free kt*64+b)
    #   col KT*B..: row0 = xsum/16; then mnr (row0 = 16*mn/s); then s as fp16
    xsum = x.astype(np.float32).sum(axis=1)
    xsb = np.zeros((128, XTM_FREE), np.float16)
    xsb[:, : KT * B] = (
        np.ascontiguousarray(x.T).reshape(KT, 128, B).transpose(1, 0, 2).reshape(128, KT * B)
    )
    xsb[0, XSUM_OFF : XSUM_OFF + B] = (xsum / 16.0).astype(np.float16)

    in_maps = []
    for c in range(NCORES):
        sl = slice(c * OSH, (c + 1) * OSH)
        wt = codes[sl, :].T                                # [IN, OSH]
        wimg = np.ascontiguousarray(
            wt.reshape(KT, 128, OSH).transpose(1, 0, 2)
        ).reshape(128, KT * OSH)
        xc = xsb.copy()
        xc[0, MNR_OFF : MNR_OFF + OSH] = (mn[sl] / s[sl] * 16.0).astype(np.float16)
        sc = s[sl].astype(np.float16)
        # scales[h*64+b, o'] = s[h*512+o']  (b-replicated)
        xc[:, SB_OFF : SB_OFF + 512] = np.broadcast_to(
            sc.reshape(2, 512)[:, None, :], (2, B, 512)
        ).reshape(128, 512)
        in_maps.append({"xsb": xc, "w8": wimg})

    global _last_in_maps
    _last_in_maps = in_maps

    nc = _build()
    res = run_bass_kernel_spmd(nc, in_maps, core_ids=list(range(NCORES)))
    # out [128, 512]: partition h*64+b, free o' -> [64, 1024]
    return np.concatenate(
        [
            res.results[c]["out"].reshape(2, B, 512).transpose(1, 0, 2).reshape(B, OSH)
            for c in range(NCORES)
        ],
        axis=1,
    ).astype(np.float16)


# revision 29
# speedup vs baseline: 1.0689x; 1.0689x over previous
"""AnyPrecisionLinear (4-bit LUT-quantized linear) on 8 TRN2 NeuronCores.

Reference computes:  out = x @ W.T,  W[o,i] = lut[o, qweight[o,i]]
  x: [64, 8192] fp16, qweight: [8192, 8192] int32 (values 0..15),
  lut: [8192, 16] fp16  ->  out: [64, 8192] fp16

Strategy (tensor-parallel along out_features, per the sharding hint):
  * Host re-encodes each row's 16-entry LUT affine into uint8 codes
    (scale s[o], offset mn[o]); ships a [8192, 1024] uint8 code shard
    per core (1 B/weight).
  * Device: streams code groups, casts uint8->fp16 on DVE+ACT in
    parallel, accumulates x @ codes.T on PE as two concurrent
    column-tiled chains (PSUM partitions 0-63 / 64-127).
  * mn[o]*xsum fold via rank-1 matmuls closing each chain; chain A
    closes first so its scale-epilogue + out-DMA overlap chain B's
    final matmuls. Host concatenates/reshapes.

Schedule (from NTFF trace analysis; best-measured variant):
  * Fine-grained single-queue DMA: 256KB first pieces so casting
    starts ~10.5us, x shipped in three k-range pieces (xsb_a covers
    groups 0-3, xsb_m 4-9, xsb_c 10-15 + xsum row + mnr + scales as
    fp16), weight tail split 2x256KB across both engines.
  * Static DVE/ACT cast assignment derived from an embedded
    event-driven list-scheduling model of arrivals.
  * Long PE warmup (~150 matmuls) bridges body-start through the
    recurring early-stream stall into the first real matmul, so the
    PE stays backlogged behind the casts for the whole kernel - it
    never idles >3us, which would trip the HAM duty-cycle throttle
    (half PE clock for a 7-10us window).
"""

import numpy as np

import concourse.bass as bass
from concourse import bacc, mybir
from concourse.bass_utils import run_bass_kernel_spmd

B, IN, OUT, NCORES = 64, 8192, 8192, 8
OSH = OUT // NCORES          # 1024 output columns per core
KT = IN // 128               # 64 contraction k-tiles of 128
NG = 16                      # 512KB cast groups (4 k-tiles each)
KPG = KT // NG               # k-tiles per group (4)
GSZ = KPG * OSH              # free elems per group (4096)

# x SBUF image pieces (free-elem offsets into the xtm tensor)
XA_KT, XM_KT = 16, 40        # xsb_a covers kt 0-15, xsb_m 16-39, xsb_c rest
XA_E = XA_KT * B                       # 1024
XM_E = (XM_KT - XA_KT) * B             # 1536
XSUM_OFF = KT * B                      # 4096 (xsum/16 row in tile KT)
MNR_OFF = (KT + 1) * B                 # 4160
SB_OFF = MNR_OFF + OSH                 # 5184
XTM_FREE = SB_OFF + 512                # 5696 fp16 elems per partition

WARMUP = 150                 # PE warmup matmuls ([128,32]x[128,128])
N_SLOTS = 8                  # rotating fp16 cast-group buffers
SPLIT_GROUPS = (0, 15)       # groups DMAd/cast as 2x256KB halves

_cached_nc = None
_last_in_maps = None


def _schedule():
    """Derive the DMA order and static DVE/ACT cast assignment from an
    event-driven model of the single-queue stream (~425 GB/s measured).
    Returns (dma_order, piece_assign, piece_order, piece_group).
    """
    XFER0, BW, GAP, SEM = 9.15, 0.43, 0.05, 0.93
    DVE_U, ACT_U, DISP = 4.60, 7.40, 0.08   # us per MB of uint8

    pieces = []          # (name, group, MB)
    for g in range(NG):
        if g in SPLIT_GROUPS:
            pieces += [(f'g{g}a', g, 0.25), (f'g{g}b', g, 0.25)]
        else:
            pieces.append((f'g{g}', g, 0.5))
    sizes = {n: mb for n, _, mb in pieces}
    sizes.update({'xsb_a': 0.26, 'xsb_m': 0.40, 'xsb_c': 0.79})

    dma_order = []
    for g in range(NG):
        dma_order += [n for n, gg, _ in pieces if gg == g]
        if g == 0:
            dma_order.append('xsb_a')
        if g == 3:
            dma_order.append('xsb_m')
        if g == 9:
            dma_order.append('xsb_c')

    t = XFER0
    arr = {}
    for n in dma_order:
        t += sizes[n] / BW + GAP
        arr[n] = t + SEM

    free = {'D': 9.3, 'A': 8.6}
    rate = {'D': DVE_U, 'A': ACT_U}
    assign, done = {}, {}
    left = [n for n, _, _ in pieces]
    while left:
        e = min(free, key=lambda k: free[k])
        cand = min(left, key=lambda n: arr[n])
        o = 'A' if e == 'D' else 'D'
        fin_e = max(free[e], arr[cand]) + DISP + sizes[cand] * rate[e]
        fin_o = max(free[o], arr[cand]) + DISP + sizes[cand] * rate[o]
        if len(left) <= 3 and fin_o < fin_e:
            e, fin_e = o, fin_o
        free[e] = fin_e
        done[cand] = fin_e
        assign[cand] = e
        left.remove(cand)
    order = sorted(done, key=lambda n: done[n])
    return dma_order, assign, order, {n: g for n, g, _ in pieces}


def _build():
    global _cached_nc
    if _cached_nc is not None:
        return _cached_nc
    from contextlib import ExitStack

    dma_order, assign, piece_order, piece_group = _schedule()

    nc = bacc.Bacc(
        "TRN2",
        target_bir_lowering=False,
        debug=False,
        enable_asserts=False,
        num_devices=NCORES,
    )
    xsb = nc.dram_tensor("xsb", [128, XTM_FREE], mybir.dt.float16, kind="ExternalInput")
    w8 = nc.dram_tensor("w8", [128, KT * OSH], mybir.dt.uint8, kind="ExternalInput")
    out = nc.dram_tensor("out", [128, 512], mybir.dt.float16, kind="ExternalOutput")

    # free-elem ranges for each DMA piece
    def piece_rng(name):
        if name == 'xsb_a':
            return 'x', 0, XA_E
        if name == 'xsb_m':
            return 'x', XA_E, XA_E + XM_E
        if name == 'xsb_c':
            return 'x', XA_E + XM_E, XTM_FREE
        g = piece_group[name]
        lo = g * GSZ
        hi = lo + GSZ
        if name.endswith('a'):
            hi = lo + GSZ // 2
        elif name.endswith('b'):
            lo = lo + GSZ // 2
        return 'w', lo, hi

    # cast ordinal per engine (sem thresholds)
    dord, aord = {}, {}
    for n in piece_order:
        if assign[n] == 'D':
            dord[n] = len(dord) + 1
        else:
            aord[n] = len(aord) + 1

    with ExitStack() as ctx:
        ec = ctx.enter_context
        dsems = {n: ec(nc.semaphore(f"d_{n}")) for n in dma_order}
        wzs = ec(nc.semaphore("wzs"))
        dcast = ec(nc.semaphore("dcast"))
        acast = ec(nc.semaphore("acast"))
        mmp = ec(nc.semaphore("mmp"))      # groups consumed; +NG: A closed, +NG+1: B
        epiA = ec(nc.semaphore("epiA"))
        epiB = ec(nc.semaphore("epiB"))
        doutA = ec(nc.semaphore("doutA"))
        doutB = ec(nc.semaphore("doutB"))
        xtm = ec(nc.sbuf_tensor("xtm", [128, XTM_FREE], mybir.dt.float16))
        w8t = ec(nc.sbuf_tensor("w8t", [128, KT * OSH], mybir.dt.uint8))
        wf = ec(nc.sbuf_tensor("wf", [128, N_SLOTS * GSZ], mybir.dt.float16))
        o16 = ec(nc.sbuf_tensor("o16", [128, 512], mybir.dt.float16))
        wz = ec(nc.sbuf_tensor("wz", [128, 128], mybir.dt.float16))
        ps1 = ec(nc.psum_tensor("ps1", [128, 512], mybir.dt.float32))
        ps2 = ec(nc.psum_tensor("ps2", [128, 512], mybir.dt.float32))
        wps = ec(nc.psum_tensor("wps", [32, 128], mybir.dt.float32))
        block = ec(nc.Block())

        # Exact ring-slot recycling thresholds: PE consumes pieces in the
        # static piece_order, so the group-completion sequence is known at
        # build time. Cast of group g (slot g % N_SLOTS) must wait until
        # group g - N_SLOTS has been fully consumed; mmp counts completed
        # groups in completion order.
        grp_completion = []
        seen = set()
        for n in piece_order:
            seen.add(n)
            g = piece_group[n]
            if g not in grp_completion and all(
                pn in seen for pn in piece_group if piece_group[pn] == g
            ):
                grp_completion.append(g)

        def slot_wait(g):
            prev = g - N_SLOTS
            if prev < 0:
                return 0
            return grp_completion.index(prev) + 1

        # groups fully consumed before the last piece (for epilogue gating)
        last_piece = piece_order[-1]
        n_grps_before_last = len(grp_completion) - (
            1 if piece_group[last_piece] == grp_completion[-1] else 0
        )

        @block.sync
        def _(sync):
            for n in dma_order:
                kind, lo, hi = piece_rng(n)
                if kind == 'x':
                    sync.dma_start(xtm[:, lo:hi], xsb[:, lo:hi]).then_inc(dsems[n], 16)
                else:
                    sync.dma_start(w8t[:, lo:hi], w8[:, lo:hi]).then_inc(dsems[n], 16)
            sync.wait_ge(epiA, 1)
            sync.dma_start(out[0:64, :], o16[0:64, :]).then_inc(doutA, 16)
            sync.wait_ge(epiB, 1)
            sync.dma_start(out[64:128, :], o16[64:128, :]).then_inc(doutB, 16)
            sync.wait_ge(doutA, 16)
            sync.wait_ge(doutB, 16)

        @block.vector
        def _(vector):
            vector.memset(wz[:, :], 0).then_inc(wzs, 1)
            for n in piece_order:
                if assign[n] != 'D':
                    continue
                g = piece_group[n]
                _, lo, hi = piece_rng(n)
                sl = (g % N_SLOTS) * GSZ + (lo - g * GSZ)
                vector.wait_ge(dsems[n], 16)
                w = slot_wait(g)
                if w:
                    vector.wait_ge(mmp, w)
                vector.tensor_copy(
                    wf[:, sl : sl + (hi - lo)], w8t[:, lo:hi]
                ).then_inc(dcast, 1)
            # epilogue: half A as soon as chain A closes (its out-DMA then
            # overlaps the half-B mul which waits for chain B)
            vector.wait_ge(mmp, n_grps_before_last + 1)   # rank1-A fired
            vector.wait_ge(dsems['xsb_c'], 16)
            vector.tensor_mul(
                o16[0:64, :], ps1[0:64, :], xtm[0:64, SB_OFF : SB_OFF + 512]
            ).then_inc(epiA, 1)
            vector.wait_ge(mmp, n_grps_before_last + 2)   # rank1-B fired
            vector.tensor_mul(
                o16[64:128, :], ps2[64:128, :], xtm[64:128, SB_OFF : SB_OFF + 512]
            ).then_inc(epiB, 1)

        @block.scalar
        def _(scalar):
            for n in piece_order:
                if assign[n] != 'A':
                    continue
                g = piece_group[n]
                _, lo, hi = piece_rng(n)
                sl = (g % N_SLOTS) * GSZ + (lo - g * GSZ)
                scalar.wait_ge(dsems[n], 16)
                w = slot_wait(g)
                if w:
                    scalar.wait_ge(mmp, w)
                scalar.copy(
                    wf[:, sl : sl + (hi - lo)], w8t[:, lo:hi]
                ).then_inc(acast, 1)

        @block.tensor
        def _(tensor):
            tensor.wait_ge(wzs, 1)
            for _ in range(WARMUP):
                tensor.matmul(wps.ap(), wz[:, 0:32], wz[:, :], start=True, stop=True)
            psA = ps1[0:64, :]
            psB = ps2[64:128, :]
            xs_lhs = xtm[:, XSUM_OFF : XSUM_OFF + B]
            # pieces in expected completion order; accumulation order is free
            consumed = set()
            waited_gates = set()
            first = True
            for n in piece_order:
                g = piece_group[n]
                _, lo, hi = piece_rng(n)
                sl = (g % N_SLOTS) * GSZ + (lo - g * GSZ)
                if assign[n] == 'D':
                    tensor.wait_ge(dcast, dord[n])
                else:
                    tensor.wait_ge(acast, aord[n])
                xgate = 'xsb_a' if g <= 3 else ('xsb_m' if g <= 9 else 'xsb_c')
                if xgate not in waited_gates:
                    tensor.wait_ge(dsems[xgate], 16)
                    waited_gates.add(xgate)
                nkt = (hi - lo) // OSH
                k0 = lo // OSH
                if n == piece_order[-1]:
                    if 'xsb_c' not in waited_gates:
                        tensor.wait_ge(dsems['xsb_c'], 16)
                    # de-interleaved tail: close chain A first (rank1-A), so
                    # the A epilogue + out-DMA overlap chain B's final mms
                    for j in range(nkt):
                        k = k0 + j
                        lhsT = xtm[:, k * B : (k + 1) * B]
                        rhs = wf[:, sl + j * OSH : sl + (j + 1) * OSH]
                        tensor.matmul(psA, lhsT, rhs[:, 0:512], start=False, stop=False)
                    tensor.matmul(
                        psA, xs_lhs, xtm[:, MNR_OFF : MNR_OFF + 512],
                        start=False, stop=True,
                    ).then_inc(mmp, 1)
                    for j in range(nkt):
                        k = k0 + j
                        lhsT = xtm[:, k * B : (k + 1) * B]
                        rhs = wf[:, sl + j * OSH : sl + (j + 1) * OSH]
                        tensor.matmul(psB, lhsT, rhs[:, 512:1024], start=False, stop=False)
                    tensor.matmul(
                        psB, xs_lhs, xtm[:, MNR_OFF + 512 : MNR_OFF + 1024],
                        start=False, stop=True,
                    ).then_inc(mmp, 1)
                    continue
                last_mm = None
                for j in range(nkt):
                    k = k0 + j
                    lhsT = xtm[:, k * B : (k + 1) * B]
                    rhs = wf[:, sl + j * OSH : sl + (j + 1) * OSH]
                    tensor.matmul(psA, lhsT, rhs[:, 0:512], start=first, stop=False)
                    last_mm = tensor.matmul(
                        psB, lhsT, rhs[:, 512:1024], start=first, stop=False
                    )
                    first = False
                consumed.add(n)
                grp_done = all(
                    (pn in consumed) for pn in piece_group if piece_group[pn] == g
                )
                if grp_done:
                    last_mm.then_inc(mmp, 1)

    nc.compile()
    _cached_nc = nc
    return nc


def kernel(x, qweight, lut):
    x = np.asarray(x, dtype=np.float16)
    qweight = np.asarray(qweight, dtype=np.int32)
    lut = np.asarray(lut, dtype=np.float16)

    # Per-row affine re-encode of the LUT into uint8 codes.
    lut32 = lut.astype(np.float32)
    mn = lut32.min(axis=1)
    mx_ = lut32.max(axis=1)
    rng = mx_ - mn
    rng[rng == 0] = 1.0
    s = (rng / 255.0).astype(np.float32)               # [OUT]
    lutcodes = np.rint((lut32 - mn[:, None]) * (255.0 / rng)[:, None]).astype(np.uint8)
    codes = np.take_along_axis(lutcodes, qweight, axis=1)  # [OUT, IN] uint8

    # x SBUF image: [128, XTM_FREE] fp16
    #   cols [0, KT*B): x tiles (partition k%128, free kt*64+b)
    #   col KT*B..: row0 = xsum/16; then mnr (row0 = 16*mn/s); then s as fp16
    xsum = x.astype(np.float32).sum(axis=1)
    xsb = np.zeros((128, XTM_FREE), np.float16)
    xsb[:, : KT * B] = (
        np.ascontiguousarray(x.T).reshape(KT, 128, B).transpose(1, 0, 2).reshape(128, KT * B)
    )
    xsb[0, XSUM_OFF : XSUM_OFF + B] = (xsum / 16.0).astype(np.float16)

    in_maps = []
    for c in range(NCORES):
        sl = slice(c * OSH, (c + 1) * OSH)
        wt = codes[sl, :].T                                # [IN, OSH]
        wimg = np.ascontiguousarray(
            wt.reshape(KT, 128, OSH).transpose(1, 0, 2)
        ).reshape(128, KT * OSH)
        xc = xsb.copy()
        xc[0, MNR_OFF : MNR_OFF + OSH] = (mn[sl] / s[sl] * 16.0).astype(np.float16)
        sc = s[sl].astype(np.float16)
        # scales[h*64+b, o'] = s[h*512+o']  (b-replicated)
        xc[:, SB_OFF : SB_OFF + 512] = np.broadcast_to(
            sc.reshape(2, 512)[:, None, :], (2, B, 512)
        ).reshape(128, 512)
        in_maps.append({"xsb": xc, "w8": wimg})

    global _last_in_maps
    _last_in_maps = in_maps

    nc = _build()
    res = run_bass_kernel_spmd(nc, in_maps, core_ids=list(range(NCORES)))
    # out [128, 512]: partition h*64+b, free o' -> [64, 1024]
    return np.concatenate(
        [
            res.results[c]["out"].reshape(2, B, 512).transpose(1, 0, 2).reshape(B, OSH)
            for c in range(NCORES)
        ],
        axis=1,
    ).astype(np.float16)
